# revision 1
# baseline (speedup 1.0000x reference)
"""Masked-linear kernel for trn2: out = x @ (mask.T * w) + b.

Full shapes: x (8192, 3072) f32, w (3072, 1536) f32, b (1536,) f32,
mask (1536, 3072) f32 -> out (8192, 1536) f32.

Strategy: 8 NeuronCores as a 4 (batch) x 2 (units) grid. Each core gets
xT (3072, 2048) bf16, w / mask.T shards (3072, 768) bf16, b shard, and
computes outT (768, 2048) f32 = (w*maskT).T @ x_shard.T + b on device:
the mask multiply runs on VectorE, the matmul on TensorE (bf16 with f32
PSUM accumulation, K split into segments accumulated in SBUF f32).
Host only slices / transposes / casts (layout) and reassembles.
"""

import os
import sys

import numpy as np
import ml_dtypes

for _p in ("/opt/trn_rl_repo",):
    if os.path.isdir(_p) and _p not in sys.path:
        sys.path.append(_p)

import concourse.bass as bass  # noqa: E402
import concourse.mybir as mybir  # noqa: E402
import concourse.tile as tile  # noqa: E402
from concourse import bacc  # noqa: E402
from concourse.bass_utils import run_bass_kernel_spmd  # noqa: E402

BF16 = ml_dtypes.bfloat16

BATCH, IN_DIM, UNITS = 8192, 3072, 1536
BW, UW = 4, 2  # batch ways x unit ways = 8 cores
BC = BATCH // BW  # 2048 batch rows per core
UC = UNITS // UW  # 768 units per core
P = 128
K_CHUNKS = IN_DIM // P  # 24
KPS = 4  # K chunks per PSUM accumulation segment
SEGS = K_CHUNKS // KPS  # 6
BT = 512  # matmul moving free dim (one PSUM bank of f32)
NB = BC // BT  # 4
NU = UC // P  # 6
N_CORES = 8

_NC_CACHE = None


def _build_module():
    nc = bacc.Bacc("TRN2", target_bir_lowering=False, debug=False)

    xT = nc.dram_tensor("xT", (IN_DIM, BC), mybir.dt.bfloat16, kind="ExternalInput")
    wp = nc.dram_tensor("wp", (IN_DIM, UC), mybir.dt.bfloat16, kind="ExternalInput")
    mp = nc.dram_tensor("mp", (IN_DIM, UC), mybir.dt.bfloat16, kind="ExternalInput")
    bp = nc.dram_tensor("bp", (P, NU), mybir.dt.float32, kind="ExternalInput")
    outT = nc.dram_tensor("outT", (UC, BC), mybir.dt.float32, kind="ExternalOutput")

    xT3 = xT.ap().rearrange("(ko p) b -> ko p b", p=P)  # [24, 128, 2048]
    wp3 = wp.ap().rearrange("(ko p) u -> ko p u", p=P)  # [24, 128, 768]
    mp3 = mp.ap().rearrange("(ko p) u -> ko p u", p=P)
    oT3 = outT.ap().rearrange("(uo p) b -> uo p b", p=P)  # [6, 128, 2048]

    with tile.TileContext(nc) as tc:
        with (
            tc.tile_pool(name="xpool", bufs=2 * KPS) as xpool,
            tc.tile_pool(name="wpool", bufs=3) as wpool,
            tc.tile_pool(name="mwpool", bufs=2 * KPS) as mwpool,
            tc.tile_pool(name="opool", bufs=1) as opool,
            tc.tile_pool(name="cpool", bufs=1) as cpool,
            tc.tile_pool(name="pspool", bufs=8, space="PSUM") as pspool,
        ):
            btile = cpool.tile([P, NU], mybir.dt.float32, name="btile")
            nc.sync.dma_start(btile[:], bp.ap())

            # persistent f32 output accumulators, one per u-chunk (6 MB)
            out_sb = [
                opool.tile([P, BC], mybir.dt.float32, name=f"osb{u}", tag=f"osb{u}")
                for u in range(NU)
            ]

            for s in range(SEGS):
                xs, mws = [], []
                for kk in range(KPS):
                    k = s * KPS + kk
                    xt = xpool.tile([P, BC], mybir.dt.bfloat16, name=f"xt{k}", tag="xt")
                    nc.sync.dma_start(xt[:], xT3[k])
                    wt = wpool.tile([P, UC], mybir.dt.bfloat16, name=f"wt{k}", tag="wt")
                    nc.sync.dma_start(wt[:], wp3[k])
                    mt = wpool.tile([P, UC], mybir.dt.bfloat16, name=f"mt{k}", tag="mt")
                    nc.sync.dma_start(mt[:], mp3[k])
                    mw = mwpool.tile(
                        [P, UC], mybir.dt.bfloat16, name=f"mw{k}", tag="mw"
                    )
                    nc.vector.tensor_mul(mw[:], wt[:], mt[:])
                    xs.append(xt)
                    mws.append(mw)

                for u in range(NU):
                    ptiles = [
                        pspool.tile(
                            [P, BT], mybir.dt.float32, name=f"ps{s}_{u}_{b}", tag="ps"
                        )
                        for b in range(NB)
                    ]
                    for kk in range(KPS):
                        lhsT = mws[kk][:, u * P : (u + 1) * P]
                        for b in range(NB):
                            nc.tensor.matmul(
                                ptiles[b][:],
                                lhsT,
                                xs[kk][:, b * BT : (b + 1) * BT],
                                start=(kk == 0),
                                stop=(kk == KPS - 1),
                            )
                    for b in range(NB):
                        osl = out_sb[u][:, b * BT : (b + 1) * BT]
                        if s == 0:
                            nc.vector.tensor_add(
                                osl,
                                ptiles[b][:],
                                btile[:, u : u + 1].to_broadcast((P, BT)),
                            )
                        else:
                            nc.vector.tensor_add(osl, osl, ptiles[b][:])
                        if s == SEGS - 1:
                            nc.sync.dma_start(oT3[u][:, b * BT : (b + 1) * BT], osl)

    nc.compile()
    return nc


def get_module():
    global _NC_CACHE
    if _NC_CACHE is None:
        _NC_CACHE = _build_module()
    return _NC_CACHE


def make_in_maps(x, w, b, mask):
    x16 = x.astype(BF16)
    w16 = w.astype(BF16)
    m16T = np.ascontiguousarray(mask.astype(BF16).T)  # (3072, 1536)
    in_maps = []
    for c in range(N_CORES):
        bc, uc = divmod(c, UW)
        in_maps.append(
            {
                "xT": np.ascontiguousarray(x16[bc * BC : (bc + 1) * BC].T),
                "wp": np.ascontiguousarray(w16[:, uc * UC : (uc + 1) * UC]),
                "mp": np.ascontiguousarray(m16T[:, uc * UC : (uc + 1) * UC]),
                "bp": np.ascontiguousarray(
                    b[uc * UC : (uc + 1) * UC].astype(np.float32).reshape(NU, P).T
                ),
            }
        )
    return in_maps


def assemble(results):
    out = np.empty((BATCH, UNITS), dtype=np.float32)
    for c in range(N_CORES):
        bc, uc = divmod(c, UW)
        out[bc * BC : (bc + 1) * BC, uc * UC : (uc + 1) * UC] = results[c]["outT"].T
    return out


def kernel(x, w, b, mask, _trace=False, _trace_kwargs=None):
    x = np.asarray(x, dtype=np.float32)
    w = np.asarray(w, dtype=np.float32)
    b = np.asarray(b, dtype=np.float32)
    mask = np.asarray(mask, dtype=np.float32)
    nc = get_module()
    in_maps = make_in_maps(x, w, b, mask)
    res = run_bass_kernel_spmd(
        nc,
        in_maps,
        core_ids=list(range(N_CORES)),
        trace=_trace,
        **(_trace_kwargs or {}),
    )
    out = assemble(res.results)
    if _trace:
        return out, res
    return out

